# revision 32
# baseline (speedup 1.0000x reference)
"""Cost-sensitive focal NLL loss on 8 Trainium2 NeuronCores.

Computes, for feature [N, C] logits and label [N] int:
    log_p = log_softmax(feature, axis=1)
    p = exp(log_p); beta = (1 - p)**2
    counts = bincount(label, C); ni = counts[label]; r = ni / N
    alpha = exp(r - 1) / r
    loss = -mean(alpha * beta[i, label[i]] * log_p[i, label[i]])

Only the label-column of log_p/beta is needed, so each core streams its
batch shard once (1 MiB DMAs, bf16 -- the 2e-2 tolerance dwarfs the
~2e-5 it costs), exponentiates on ScalarE and row-reduces on VectorE.
Everything that depends only on `label` (class histogram, gather
offsets -> the label logits themselves, per-row counts) is integer
bookkeeping prepared host-side during sharding; the device computes all
the float math: sumexp over [N, C], log-softmax at the label, focal
beta, cost weight alpha, and one partial sum per core.  The host adds
the 8 partials and divides by -N.
"""

import contextlib
import ctypes
import os
import sys
import types

import ml_dtypes
import numpy as np

import concourse.bacc as bacc
import concourse.mybir as mybir
import concourse.tile as tile
from concourse.bass_utils import run_bass_kernel_spmd


def _install_ntff_shim():
    """run_bass_kernel_spmd(trace=True) needs antenv.axon_hooks; some agent
    images lack it.  Recreate trn_boot.py's ctypes NTFF hook so profiling
    works either way (no-op when the real module exists)."""
    try:
        import antenv.axon_hooks  # noqa: F401

        return
    except ImportError:
        pass
    try:
        import antenv
    except ImportError:
        return

    mod = types.ModuleType("antenv.axon_hooks")
    mod._hook = None
    mod.set_axon_ntff_profile_hook = lambda h: setattr(mod, "_hook", h)
    mod.get_axon_ntff_profile_hook = lambda: mod._hook
    sys.modules["antenv.axon_hooks"] = mod
    antenv.axon_hooks = mod

    so_path = "/opt/axon/libaxon_pjrt.so"
    if not os.path.exists(so_path):
        return
    try:
        lib = ctypes.CDLL(so_path)
    except OSError:
        return
    if not hasattr(lib, "axon_start_nrt_profile"):
        return
    lib.axon_start_nrt_profile.argtypes = [
        ctypes.POINTER(ctypes.c_int64),
        ctypes.c_size_t,
    ]
    lib.axon_start_nrt_profile.restype = ctypes.c_int64
    lib.axon_stop_nrt_profile.argtypes = [ctypes.c_char_p]
    lib.axon_stop_nrt_profile.restype = ctypes.c_int64

    @contextlib.contextmanager
    def _hook(output_dir, device_ids):
        import jax

        jax.devices()
        if device_ids:
            ids = (ctypes.c_int64 * len(device_ids))(*device_ids)
            rc = lib.axon_start_nrt_profile(ids, len(device_ids))
        else:
            rc = lib.axon_start_nrt_profile(None, 0)
        if rc != 0:
            raise RuntimeError(f"axon_start_nrt_profile rc={rc}")
        try:
            yield
        finally:
            lib.axon_stop_nrt_profile(str(output_dir).encode())

    mod.set_axon_ntff_profile_hook(_hook)


_install_ntff_shim()

N_CORES = 8
N = 16384
C = 1000
P = 128
ROWS = N // N_CORES          # 2048 rows per core
T = ROWS // P                # 16 row-tiles per core
# row-tiles per DMA: small transfers first so ScalarE starts early, then
# uniform 0.5 MiB pairs that land faster than ScalarE consumes them (no
# mid-stream gaps from big-transfer completion latency).  Tile 14 sums via
# ScalarE accum_out to offload VectorE; the last tile reduces on VectorE so
# that reduce overlaps the Ln table load on ScalarE.
DMA_GROUPS = (1, 1, 2, 4, 4, 1, 1, 1, 1)
ACCUM_TILES = (13, 14, 15)   # row-sum via fused accum_out on ScalarE
LN_SPLIT = 13                # lse computed in two pieces (see below)
assert sum(DMA_GROUPS) == T

FP = mybir.dt.float32
BF = mybir.dt.bfloat16

LAST_RESULTS = None  # BassKernelResults of the most recent run (for profiling)


def build_program(dump_debug: bool = False):
    nc = bacc.Bacc(
        "TRN2",
        target_bir_lowering=False,
        debug=False,
        enable_asserts=False,
        num_devices=N_CORES,
    )

    feature = nc.dram_tensor("feature", [ROWS, C], BF, kind="ExternalInput")
    xg_cm = nc.dram_tensor("xg_cm", [P, T], FP, kind="ExternalInput")
    cnt_cm = nc.dram_tensor("cnt_cm", [P, T], FP, kind="ExternalInput")
    out = nc.dram_tensor("out", [1, 1], FP, kind="ExternalOutput")
    dbg = {}
    if dump_debug:
        for nm in ("d_s", "d_u", "d_alpha"):
            dbg[nm] = nc.dram_tensor(nm, [P, T], FP, kind="ExternalOutput")

    with tile.TileContext(nc) as tc:
        with (
            tc.tile_pool(name="const", bufs=1) as const_pool,
            tc.tile_pool(name="feat", bufs=6) as feat_pool,
            tc.tile_pool(name="escr", bufs=4) as escr_pool,
            tc.tile_pool(name="small", bufs=1) as small_pool,
        ):
            neg1_col = const_pool.tile([P, 1], FP)
            nc.vector.memset(neg1_col[:], -1.0)

            # xg first on the sync queue (8 KB, lands ~immediately) because
            # the scheduler likes to park ex=Exp(xg) at the head of ScalarE's
            # stream; cnt via the idle gpsimd SWDGE queue
            xg = small_pool.tile([P, T], FP)
            nc.sync.dma_start(xg[:], xg_cm.ap())
            cnt = small_pool.tile([P, T], FP)
            nc.gpsimd.dma_start(cnt[:], cnt_cm.ap())

            # ---- stream feature tiles: s[row] = sum_c exp(feature[row, c])
            # One ACT exp per landed DMA group.  Row-sums: per-tile [P, C]
            # reduces on VectorE for the leading groups, fused accum_out on
            # ScalarE for the trailing single-tile groups.
            feat_t = feature.ap().rearrange("(t p) c -> p t c", p=P)
            s_col = small_pool.tile([P, T], FP)
            t0 = 0
            for gi, g in enumerate(DMA_GROUPS):
                ft = feat_pool.tile([P, g * C], BF, name="ft")
                # tile 0 comes via the scalar HWDGE queue: ScalarE dispatches
                # it itself before its table load, so the first exp starts
                # ~1.5us earlier than waiting on the sync queue's head
                dma_eng = nc.scalar if gi == 0 else nc.sync
                dma_eng.dma_start(
                    ft[:].rearrange("p (g c) -> p g c", g=g),
                    feat_t[:, t0 : t0 + g, :],
                )
                esc = escr_pool.tile([P, g * C], BF, name="esc")
                accum = t0 in ACCUM_TILES
                nc.scalar.activation(
                    esc[:],
                    ft[:],
                    mybir.ActivationFunctionType.Exp,
                    accum_out=s_col[:, t0 : t0 + 1] if accum else None,
                )
                if not accum:
                    for j in range(g):
                        nc.vector.tensor_reduce(
                            s_col[:, t0 + j : t0 + j + 1],
                            esc[:, j * C : (j + 1) * C],
                            axis=mybir.AxisListType.X,
                            op=mybir.AluOpType.add,
                        )
                t0 += g

                if gi == 2:
                    # per-row alpha = exp(r-1)/r, r = cnt/N (no Ln needed);
                    # emitted mid-stream: its inputs landed long ago, and this
                    # placement keeps the scheduler from parking it in front
                    # of the first stream exp on ScalarE
                    e1 = small_pool.tile([P, T], FP)
                    nc.scalar.activation(
                        e1[:],
                        cnt[:],
                        mybir.ActivationFunctionType.Exp,
                        bias=neg1_col[:],
                        scale=1.0 / N,
                    )
                    rc = small_pool.tile([P, T], FP)
                    nc.vector.reciprocal(rc[:], cnt[:])
                    alpha = small_pool.tile([P, T], FP)  # exp(r-1)*N*(1/cnt)
                    nc.vector.scalar_tensor_tensor(
                        alpha[:],
                        in0=e1[:],
                        scalar=float(N),
                        in1=rc[:],
                        op0=mybir.AluOpType.mult,
                        op1=mybir.AluOpType.mult,
                    )
                    ex = small_pool.tile([P, T], FP)
                    nc.scalar.activation(
                        ex[:], xg[:], mybir.ActivationFunctionType.Exp
                    )

            # ---- per-row tail ----
            # lse in two pieces: the first covers the accum tiles, whose
            # s_col columns come from ScalarE itself -- so the Ln table load
            # (which inherits the first Ln's wait) is gated only on ScalarE's
            # own pipeline, not on VectorE's trailing reduces.
            lse = small_pool.tile([P, T], FP)
            nc.scalar.activation(
                lse[:, LN_SPLIT:T],
                s_col[:, LN_SPLIT:T],
                mybir.ActivationFunctionType.Ln,
            )
            nc.scalar.activation(
                lse[:, 0:LN_SPLIT],
                s_col[:, 0:LN_SPLIT],
                mybir.ActivationFunctionType.Ln,
            )
            rs = small_pool.tile([P, T], FP)
            nc.vector.reciprocal(rs[:], s_col[:])

            logp = small_pool.tile([P, T], FP)
            nc.vector.tensor_tensor(
                logp[:], xg[:], lse[:], op=mybir.AluOpType.subtract
            )
            pp = small_pool.tile([P, T], FP)  # p = exp(x)/s
            nc.vector.tensor_tensor(pp[:], ex[:], rs[:], op=mybir.AluOpType.mult)

            # u = (p-1)^2 * logp * alpha  ==  ((p-1)*logp) * ((p-1)*alpha)
            t1 = small_pool.tile([P, T], FP)
            nc.vector.scalar_tensor_tensor(
                t1[:],
                in0=pp[:],
                scalar=-1.0,
                in1=logp[:],
                op0=mybir.AluOpType.add,
                op1=mybir.AluOpType.mult,
            )
            t2 = small_pool.tile([P, T], FP)
            nc.vector.scalar_tensor_tensor(
                t2[:],
                in0=pp[:],
                scalar=-1.0,
                in1=alpha[:],
                op0=mybir.AluOpType.add,
                op1=mybir.AluOpType.mult,
            )
            u = small_pool.tile([P, T], FP)
            nc.vector.tensor_tensor(u[:], t1[:], t2[:], op=mybir.AluOpType.mult)

            # partial = sum_{p,t} u  (row-reduce on DVE, cross-partition on
            # GpSimd -- keeps the TensorEngine entirely out of the program)
            us = small_pool.tile([P, 1], FP)
            nc.vector.tensor_reduce(
                us[:], u[:], axis=mybir.AxisListType.X, op=mybir.AluOpType.add
            )
            fin_sb = small_pool.tile([1, 1], FP)
            nc.gpsimd.tensor_reduce(
                fin_sb[:], us[:], axis=mybir.AxisListType.C, op=mybir.AluOpType.add
            )
            nc.sync.dma_start(out.ap(), fin_sb[:])

            if dump_debug:
                nc.sync.dma_start(dbg["d_s"].ap(), s_col[:])
                nc.sync.dma_start(dbg["d_u"].ap(), u[:])
                nc.sync.dma_start(dbg["d_alpha"].ap(), alpha[:])

    nc.compile()
    return nc


_NC_CACHE = None


def _get_nc():
    global _NC_CACHE
    if _NC_CACHE is None:
        _NC_CACHE = build_program()
    return _NC_CACHE


def _to_bf16(a: np.ndarray) -> np.ndarray:
    """fp32 -> bf16 (round-to-nearest-even; ml_dtypes ships with jax)."""
    return a.astype(ml_dtypes.bfloat16)


def kernel(feature: np.ndarray, label: np.ndarray) -> np.ndarray:
    global LAST_RESULTS
    feature = np.ascontiguousarray(np.asarray(feature, dtype=np.float32))
    label = np.asarray(label)
    assert feature.shape == (N, C), feature.shape
    assert label.shape == (N,), label.shape

    lab32 = label.astype(np.int32)
    counts = np.bincount(lab32, minlength=C).astype(np.float32)  # global
    picked = feature[np.arange(N), lab32]  # label logits, fp32

    in_maps = []
    for k in range(N_CORES):
        fshard = feature[k * ROWS : (k + 1) * ROWS]
        lshard = lab32[k * ROWS : (k + 1) * ROWS]
        # column-major: [p, t] = row t*P + p, matching row-tile partitions
        lab_cm = lshard.reshape(T, P).T
        xg_cm = picked[k * ROWS : (k + 1) * ROWS].reshape(T, P).T
        in_maps.append(
            {
                "feature": np.ascontiguousarray(_to_bf16(fshard)),
                "xg_cm": np.ascontiguousarray(xg_cm),
                "cnt_cm": np.ascontiguousarray(counts[lab_cm]),
            }
        )

    nc = _get_nc()
    trace = bool(int(os.environ.get("KERNEL_TRACE", "0")))
    res = run_bass_kernel_spmd(
        nc,
        in_maps,
        core_ids=list(range(N_CORES)),
        trace=trace,
    )
    LAST_RESULTS = res

    total = 0.0
    for k in range(N_CORES):
        total += float(res.results[k]["out"][0, 0])
    return np.float32(-total / N)
